# revision 18
# baseline (speedup 1.0000x reference)
"""Int4 group-quantized dense MLP matmul on 8 Trainium2 NeuronCores.

Computes out[b,s,n] = sum_k x[b,s,k] * W[n,k] where W is dequantized from
packed int4 (two nibbles per byte, per-128-group fp16 scales).

Strategy (tensor-parallel over N):
  - N=11008 output features sharded 1376 per core across 8 cores.
  - Host does LAYOUT ONLY: transpose x to k-major (with a per-k-tile
    nibble-parity permutation so device-side dequant never needs a
    transpose), transpose/duplicate the packed weight bytes, replicate
    scales. All arithmetic (nibble extract, -8, x scale, matmul) runs on
    device.
  - Device per core: dequantize W^T into SBUF-resident fp16 tiles
    [128 k x 1376 n] (lo nibbles -> partitions 0-63, hi -> 64-127), then
    for each 128-token tile accumulate 32 k-tile matmuls into PSUM fp32
    and write fp16 output rows.
"""

import numpy as np

B, S, K, N = 4, 2048, 4096, 11008
T = B * S                      # 8192 tokens
P = 128                        # partitions
KT = K // P                    # 32 k-tiles (each is one quant group)
NCORES = 8
NC_N = N // NCORES             # 1376 features per core
TOK_CHUNK = 256                # tokens per x DMA chunk
MM_FREE = 512                  # max moving free dim per matmul (PSUM bank)


# ---------------------------------------------------------------------------
# walrus in this container accepts only ONE sync wait per instruction;
# split extras onto same-engine NoOps placed immediately before.
def _legalize_multi_waits(nc, max_waits=1):
    from concourse import mybir

    n_fixed = 0
    for f in nc.m.functions:
        for bb in f.blocks:
            insts = bb.instructions
            i = 0
            while i < len(insts):
                inst = insts[i]
                si = inst.sync_info
                if si is not None and si.on_wait and len(si.on_wait) > max_waits:
                    waits = list(si.on_wait)
                    extra, keep = waits[:-max_waits], waits[-max_waits:]
                    chain = []
                    for j in range(0, len(extra), max_waits):
                        chunk = extra[j : j + max_waits]
                        chain.append(
                            mybir.InstNoOp(
                                name=f"{inst.name}-waitsplit-{j}",
                                engine=inst.engine,
                                bass_nofuse=True,
                                sync_info=mybir.SyncInfo(on_wait=chunk, on_update=[]),
                            )
                        )
                    si.on_wait = keep
                    for k, nop in enumerate(chain):
                        insts.insert(i + k, nop)
                    i += len(chain)
                    n_fixed += 1
                i += 1
    return n_fixed


def _install_ntff_shim():
    """Make trace=True work: register the NTFF profile hook that the agent
    image's antenv lacks, and keep artifacts local."""
    import sys, types

    try:
        import antenv.axon_hooks  # noqa: F401

        return
    except ImportError:
        pass
    try:
        from trn_agent_boot.trn_boot import _ntff_profile_via_ctypes

        hook = _ntff_profile_via_ctypes("/opt/axon/libaxon_pjrt.so")
    except Exception:
        hook = None
    mod = types.ModuleType("antenv.axon_hooks")
    mod.get_axon_ntff_profile_hook = lambda: hook
    mod.set_axon_ntff_profile_hook = lambda h: None
    sys.modules["antenv.axon_hooks"] = mod

    import concourse.bass_utils as bu

    bu.upload_artifacts = lambda tmpdir: "local://" + str(tmpdir)


# ---------------------------------------------------------------------------
def build_nc(t=T, k=K, nc_n=NC_N, tok_chunk=TOK_CHUNK):
    """Build the per-core Bass program (same NEFF on all cores; per-core
    inputs differ). Inputs: xh [t/tok_chunk*ksplit, 128, ktn_per*tok_chunk]
    fp16 (per-partition-contiguous sub-tiles), wpk [kt, 128, nc_n] u8,
    scl [kt, 128, nc_n] fp16. Output: out [t, nc_n] fp16."""
    import concourse.bass as bass
    import concourse.tile as tile
    from concourse import mybir

    kt_n = k // P
    assert t % tok_chunk == 0 and tok_chunk % P == 0
    n_splits = [
        (n0, min(MM_FREE, nc_n - n0)) for n0 in range(0, nc_n, MM_FREE)
    ]

    nc = bass.Bass()
    # const AP for the ACT bias (-8): same pattern as Bass.__init__ consts
    _c = nc.alloc_sbuf_tensor("const-float32-m8", [P, 1], mybir.dt.float32)
    nc.gpsimd.memset(_c.ap(), -8.0)
    nc.const_aps.aps[(mybir.dt.float32, -8.0)] = _c.ap()
    nc.all_engine_barrier()

    ksplit = 4 if kt_n % 4 == 0 else 1
    ktn_per = kt_n // ksplit

    xh = nc.declare_dram_parameter(
        "xh",
        [t // tok_chunk * ksplit, P, ktn_per * tok_chunk],
        mybir.dt.float16,
        isOutput=False,
    )
    wpk = nc.declare_dram_parameter(
        "wpk", [kt_n, 64, nc_n], mybir.dt.uint8, isOutput=False
    )
    scl = nc.declare_dram_parameter(
        "scl", [kt_n, P, nc_n], mybir.dt.float16, isOutput=False
    )
    out = nc.declare_dram_parameter("out", [t, nc_n], mybir.dt.float16, isOutput=True)

    with tile.TileContext(nc) as tc:
        with (
            tc.tile_pool(name="wt", bufs=1) as wt_pool,
            tc.tile_pool(name="wsb", bufs=3) as wsb_pool,
            tc.tile_pool(name="ssb", bufs=3) as ssb_pool,
            tc.tile_pool(name="xt", bufs=3) as x_pool,
            tc.tile_pool(name="osb", bufs=3) as out_pool,
            tc.tile_pool(name="psum", bufs=2, space="PSUM") as psum_pool,
            tc.tile_pool(name="pse", bufs=1, space="PSUM") as pse_pool,
        ):
            def load_x_sub(c0, s):
                xs = x_pool.tile(
                    [P, ktn_per, tok_chunk], mybir.dt.float16,
                    tag=f"xt{s}", name=f"xt{s}_{c0}",
                )
                nc.sync.dma_start(
                    xs[:],
                    xh[c0 // tok_chunk * ksplit + s].rearrange(
                        "p (kk tt) -> p kk tt", kk=ktn_per
                    ),
                )
                return xs

            def load_x_chunk(c0):
                """x chunk as `ksplit` sub-tiles (each 4KB-contiguous per
                partition in DRAM) so the first k-tiles land early."""
                return [load_x_sub(c0, s) for s in range(ksplit)]

            def alloc_psums(ts_abs):
                return [
                    psum_pool.tile(
                        [P, MM_FREE], mybir.dt.float32,
                        tag=f"ps{j}", name=f"ps{j}_{ts_abs}",
                    )
                    for j in range(len(n_splits))
                ]

            def emit_mms(psums, xsubs, ts, kt):
                lhsT = xsubs[kt // ktn_per][
                    :, kt % ktn_per, ts * P : (ts + 1) * P
                ]
                for j, (n0, w) in enumerate(n_splits):
                    nc.tensor.matmul(
                        psums[j][:, :w],
                        lhsT,
                        wt_tiles[kt][:, n0 : n0 + w],
                        start=(kt == 0),
                        stop=(kt == kt_n - 1),
                    )

            def emit_tail(psums, r0):
                osb = out_pool.tile(
                    [P, nc_n], mybir.dt.float16, tag="osb", name=f"osb{r0}"
                )
                for j, (n0, w) in enumerate(n_splits):
                    nc.scalar.copy(osb[:, n0 : n0 + w], psums[j][:, :w])
                nc.sync.dma_start(out[r0 : r0 + P, :], osb[:])

            # ---- dequant prologue: W^T tiles resident in SBUF.
            # The dequant stream is DMA-rate-bound (~2us per kt), so the
            # PE chases it kt-outer with the MMs of the first THREE
            # 128-token tiles: t0/t1 (chunk 0) fully on the main psum
            # gens, t2 (chunk 1) n-splits 0,1 on the 2 spare PSUM banks
            # (its 352-wide split runs as a burst right after).  x sub
            # loads are staggered so early wt DMAs aren't queued behind
            # x bytes that aren't needed until later k-tiles.
            xt_tiles = {0: [None] * ksplit, tok_chunk: [None] * ksplit}
            # sub s of chunk c covers k-tiles 8s..8s+7; its MMs start at
            # kt=8s, so issue its DMA a few k-tiles ahead.  Chase MM
            # emission lags one kt so kt0/kt1 weight DMAs queue ahead of
            # the first x bytes.
            x_stagger = {
                1: [(0, 0), (tok_chunk, 0)],
                4: [(0, 1)], 5: [(tok_chunk, 1)],
                12: [(0, 2)], 13: [(tok_chunk, 2)],
                20: [(0, 3)], 21: [(tok_chunk, 3)],
            }
            wt_tiles = []
            pro_full = [0, 1]          # t-tiles 0,1 from chunk 0
            pro_psums = [alloc_psums(ts) for ts in pro_full]
            pse = [
                pse_pool.tile(
                    [P, MM_FREE], mybir.dt.float32,
                    tag=f"pse{j}", name=f"pse{j}",
                )
                for j in range(2)
            ]

            def emit_chase(kt):
                for i, ts in enumerate(pro_full):
                    emit_mms(pro_psums[i], xt_tiles[0], ts, kt)
                # partial t2 chase: n-splits 0,1 into the spare banks
                lhsT2 = xt_tiles[tok_chunk][kt // ktn_per][
                    :, kt % ktn_per, 0:P
                ]
                for j in range(2):
                    n0, w = n_splits[j]
                    nc.tensor.matmul(
                        pse[j][:, :w], lhsT2, wt_tiles[kt][:, n0 : n0 + w],
                        start=(kt == 0), stop=(kt == kt_n - 1),
                    )

            for kt in range(kt_n):
                wsb = wsb_pool.tile([P, nc_n], mybir.dt.uint8, tag="wsb")
                nc.sync.dma_start(wsb[0:64, :], wpk[kt])
                ssb = ssb_pool.tile([P, nc_n], mybir.dt.float16, tag="ssb")
                nc.sync.dma_start(ssb[:], scl[kt])
                wt = wt_pool.tile([P, nc_n], mybir.dt.float16, tag=f"wt{kt}")
                # DMA brings the packed bytes once (64 partitions); the DVE
                # spreads hi nibbles to partitions 64-127 via cross-quadrant
                # 32-partition writes, then masks lo in place.  Word-wise
                # u32 ops process 4 bytes per lane-cycle; the 0x0F0F0F0F
                # mask clears cross-byte bits.
                w32 = wsb[:].bitcast(mybir.dt.uint32)
                for q in range(2):
                    nc.vector.tensor_scalar(
                        w32[64 + 32 * q : 96 + 32 * q, :],
                        w32[32 * q : 32 * (q + 1), :],
                        4, 0x0F0F0F0F,
                        mybir.AluOpType.logical_shift_right,
                        mybir.AluOpType.bitwise_and,
                    )
                nc.vector.tensor_scalar(
                    w32[0:64, :], w32[0:64, :], 0x0F0F0F0F, None,
                    mybir.AluOpType.bitwise_and,
                )
                # (nibble - 8) cast to fp16 on ACT
                nc.scalar.activation(
                    wt[:], wsb[:], mybir.ActivationFunctionType.Identity,
                    bias=-8.0, scale=1.0,
                )
                # x per-(n, group) scale on DVE
                nc.vector.tensor_tensor(
                    wt[:], wt[:], ssb[:], mybir.AluOpType.mult
                )
                wt_tiles.append(wt)
                for c0s, s in x_stagger.get(kt, ()):
                    xt_tiles[c0s][s] = load_x_sub(c0s, s)
                if kt >= 1:
                    emit_chase(kt - 1)
            emit_chase(kt_n - 1)

            for i, ts in enumerate(pro_full):
                emit_tail(pro_psums[i], ts * P)
            # t2's remaining 352-wide split: full-speed burst (wt resident)
            ps2_t2 = psum_pool.tile(
                [P, MM_FREE], mybir.dt.float32, tag="ps2", name="ps2_t2"
            )
            n0_2, w_2 = n_splits[2]
            for kt in range(kt_n):
                nc.tensor.matmul(
                    ps2_t2[:, :w_2],
                    xt_tiles[tok_chunk][kt // ktn_per][:, kt % ktn_per, 0:P],
                    wt_tiles[kt][:, n0_2 : n0_2 + w_2],
                    start=(kt == 0), stop=(kt == kt_n - 1),
                )
            emit_tail([pse[0], pse[1], ps2_t2], tok_chunk)

            # ---- main loop: x^T chunks x W^T -> out rows ----
            for c0 in range(0, t, tok_chunk):
                if c0 in xt_tiles:
                    xsubs = xt_tiles.pop(c0)
                else:
                    xsubs = load_x_chunk(c0)
                for ts in range(tok_chunk // P):
                    if (c0, ts) in ((0, 0), (0, 1), (tok_chunk, 0)):
                        continue  # handled in the prologue
                    psums = alloc_psums(c0 + ts * P)
                    for kt in range(kt_n):
                        emit_mms(psums, xsubs, ts, kt)
                    emit_tail(psums, c0 + ts * P)
    return nc


# ---------------------------------------------------------------------------
def pack_inputs(x, weight_packed, scales, t=T, k=K, nc_n=NC_N, ncores=NCORES):
    """Host-side layout prep (transpose/permute only)."""
    x = np.asarray(x, dtype=np.float16).reshape(t, k)
    wp = np.asarray(weight_packed, dtype=np.uint8)
    sc = np.asarray(scales, dtype=np.float16)
    kt_n = k // P
    ksplit = 4 if kt_n % 4 == 0 else 1
    ktn_per = kt_n // ksplit
    n_chunks = t // TOK_CHUNK

    # xTp[kt*128 + par*64 + j, tt] = x[tt, kt*128 + 2j + par]  (nibble-parity
    # permutation so device-side dequant never needs a transpose), then
    # regroup into per-(chunk, ksplit) sub-tiles that are contiguous per
    # partition: xh[(c*ksplit + s), p, kk*TOK_CHUNK + tt]
    #   = xTp[(s*ktn_per + kk)*128 + p, c*TOK_CHUNK + tt]
    xTp = x.reshape(t, kt_n, 64, 2).transpose(1, 3, 2, 0).reshape(k, t)
    xh = np.ascontiguousarray(
        xTp.reshape(ksplit, ktn_per, P, n_chunks, TOK_CHUNK)
        .transpose(3, 0, 2, 1, 4)
        .reshape(n_chunks * ksplit, P, ktn_per * TOK_CHUNK)
    )

    in_maps = []
    for c in range(ncores):
        n0 = c * nc_n
        wpT = wp[n0 : n0 + nc_n].T  # [k/2, nc_n]
        wpk = np.ascontiguousarray(wpT.reshape(kt_n, 64, nc_n))
        sclT = sc[n0 : n0 + nc_n].T  # [kt_n, nc_n]
        scl = np.ascontiguousarray(
            np.broadcast_to(sclT[:, None, :], (kt_n, P, nc_n))
        )
        in_maps.append({"xh": xh, "wpk": wpk, "scl": scl})
    return in_maps


def run(x, weight_packed, scales, trace=False):
    _install_ntff_shim()
    from concourse.bass_utils import run_bass_kernel_spmd

    nc = build_nc()
    _legalize_multi_waits(nc, max_waits=1)
    in_maps = pack_inputs(x, weight_packed, scales)
    # transient NRT device errors (NRT_EXEC_UNIT_UNRECOVERABLE) have been
    # observed to clear on retry; back off briefly between attempts.
    import time as _time

    last_exc = None
    for attempt in range(4):
        try:
            res = run_bass_kernel_spmd(
                nc, in_maps, core_ids=list(range(NCORES)), trace=trace
            )
            break
        except Exception as e:
            last_exc = e
            _time.sleep(15 * (attempt + 1))
    else:
        raise last_exc
    parts = [res.results[c]["out"] for c in range(NCORES)]
    full = np.concatenate(parts, axis=1).reshape(B, S, N)
    return full, res


def kernel(x, weight_packed, scales):
    full, _ = run(x, weight_packed, scales, trace=False)
    return full


if __name__ == "__main__":
    rng = np.random.default_rng(0)
    x = rng.standard_normal((B, S, K)).astype(np.float16)
    wp = rng.integers(0, 256, (N, K // 2)).astype(np.uint8)
    sc = (rng.random((N, K // KT)).astype(np.float16) * np.float16(0.1))
    out = kernel(x, wp, sc)
    print(out.shape, out.dtype)



# revision 19
# speedup vs baseline: 1.0125x; 1.0125x over previous
"""Int4 group-quantized dense MLP matmul on 8 Trainium2 NeuronCores.

Computes out[b,s,n] = sum_k x[b,s,k] * W[n,k] where W is dequantized from
packed int4 (two nibbles per byte, per-128-group fp16 scales).

Strategy (tensor-parallel over N):
  - N=11008 output features sharded 1376 per core across 8 cores.
  - Host does LAYOUT ONLY: transpose x to k-major (with a per-k-tile
    nibble-parity permutation so device-side dequant never needs a
    transpose), transpose/duplicate the packed weight bytes, replicate
    scales. All arithmetic (nibble extract, -8, x scale, matmul) runs on
    device.
  - Device per core: dequantize W^T into SBUF-resident fp16 tiles
    [128 k x 1376 n] (lo nibbles -> partitions 0-63, hi -> 64-127), then
    for each 128-token tile accumulate 32 k-tile matmuls into PSUM fp32
    and write fp16 output rows.
"""

import numpy as np

B, S, K, N = 4, 2048, 4096, 11008
T = B * S                      # 8192 tokens
P = 128                        # partitions
KT = K // P                    # 32 k-tiles (each is one quant group)
NCORES = 8
NC_N = N // NCORES             # 1376 features per core
TOK_CHUNK = 256                # tokens per x DMA chunk
MM_FREE = 512                  # max moving free dim per matmul (PSUM bank)


# ---------------------------------------------------------------------------
# walrus in this container accepts only ONE sync wait per instruction;
# split extras onto same-engine NoOps placed immediately before.
def _legalize_multi_waits(nc, max_waits=1):
    from concourse import mybir

    n_fixed = 0
    for f in nc.m.functions:
        for bb in f.blocks:
            insts = bb.instructions
            i = 0
            while i < len(insts):
                inst = insts[i]
                si = inst.sync_info
                if si is not None and si.on_wait and len(si.on_wait) > max_waits:
                    waits = list(si.on_wait)
                    extra, keep = waits[:-max_waits], waits[-max_waits:]
                    chain = []
                    for j in range(0, len(extra), max_waits):
                        chunk = extra[j : j + max_waits]
                        chain.append(
                            mybir.InstNoOp(
                                name=f"{inst.name}-waitsplit-{j}",
                                engine=inst.engine,
                                bass_nofuse=True,
                                sync_info=mybir.SyncInfo(on_wait=chunk, on_update=[]),
                            )
                        )
                    si.on_wait = keep
                    for k, nop in enumerate(chain):
                        insts.insert(i + k, nop)
                    i += len(chain)
                    n_fixed += 1
                i += 1
    return n_fixed


def _install_ntff_shim():
    """Make trace=True work: register the NTFF profile hook that the agent
    image's antenv lacks, and keep artifacts local."""
    import sys, types

    try:
        import antenv.axon_hooks  # noqa: F401

        return
    except ImportError:
        pass
    try:
        from trn_agent_boot.trn_boot import _ntff_profile_via_ctypes

        hook = _ntff_profile_via_ctypes("/opt/axon/libaxon_pjrt.so")
    except Exception:
        hook = None
    mod = types.ModuleType("antenv.axon_hooks")
    mod.get_axon_ntff_profile_hook = lambda: hook
    mod.set_axon_ntff_profile_hook = lambda h: None
    sys.modules["antenv.axon_hooks"] = mod

    import concourse.bass_utils as bu

    bu.upload_artifacts = lambda tmpdir: "local://" + str(tmpdir)


# ---------------------------------------------------------------------------
def build_nc(t=T, k=K, nc_n=NC_N, tok_chunk=TOK_CHUNK):
    """Build the per-core Bass program (same NEFF on all cores; per-core
    inputs differ). Inputs: xh [t/tok_chunk*ksplit, 128, ktn_per*tok_chunk]
    fp16 (per-partition-contiguous sub-tiles), wpk [kt, 128, nc_n] u8,
    scl [kt, 128, nc_n] fp16. Output: out [t, nc_n] fp16."""
    import concourse.bass as bass
    import concourse.tile as tile
    from concourse import mybir

    kt_n = k // P
    assert t % tok_chunk == 0 and tok_chunk % P == 0
    n_splits = [
        (n0, min(MM_FREE, nc_n - n0)) for n0 in range(0, nc_n, MM_FREE)
    ]

    nc = bass.Bass()
    # const AP for the ACT bias (-8): same pattern as Bass.__init__ consts
    _c = nc.alloc_sbuf_tensor("const-float32-m8", [P, 1], mybir.dt.float32)
    nc.gpsimd.memset(_c.ap(), -8.0)
    nc.const_aps.aps[(mybir.dt.float32, -8.0)] = _c.ap()
    nc.all_engine_barrier()

    ksplit = 4 if kt_n % 4 == 0 else 1
    ktn_per = kt_n // ksplit

    xh = nc.declare_dram_parameter(
        "xh",
        [t // tok_chunk * ksplit, P, ktn_per * tok_chunk],
        mybir.dt.float16,
        isOutput=False,
    )
    wpk = nc.declare_dram_parameter(
        "wpk", [kt_n, P, nc_n], mybir.dt.uint8, isOutput=False
    )
    scl = nc.declare_dram_parameter(
        "scl", [kt_n, P, nc_n], mybir.dt.float16, isOutput=False
    )
    out = nc.declare_dram_parameter("out", [t, nc_n], mybir.dt.float16, isOutput=True)

    with tile.TileContext(nc) as tc:
        with (
            tc.tile_pool(name="wt", bufs=1) as wt_pool,
            tc.tile_pool(name="wsb", bufs=5) as wsb_pool,
            tc.tile_pool(name="ssb", bufs=5) as ssb_pool,
            tc.tile_pool(name="xt", bufs=4) as x_pool,
            tc.tile_pool(name="osb", bufs=3) as out_pool,
            tc.tile_pool(name="psum", bufs=2, space="PSUM") as psum_pool,
            tc.tile_pool(name="pse", bufs=1, space="PSUM") as pse_pool,
        ):
            def load_x_sub(c0, s):
                xs = x_pool.tile(
                    [P, ktn_per, tok_chunk], mybir.dt.float16,
                    tag=f"xt{s}", name=f"xt{s}_{c0}",
                )
                nc.sync.dma_start(
                    xs[:],
                    xh[c0 // tok_chunk * ksplit + s].rearrange(
                        "p (kk tt) -> p kk tt", kk=ktn_per
                    ),
                )
                return xs

            def load_x_chunk(c0):
                """x chunk as `ksplit` sub-tiles (each 4KB-contiguous per
                partition in DRAM) so the first k-tiles land early."""
                return [load_x_sub(c0, s) for s in range(ksplit)]

            def alloc_psums(ts_abs):
                return [
                    psum_pool.tile(
                        [P, MM_FREE], mybir.dt.float32,
                        tag=f"ps{j}", name=f"ps{j}_{ts_abs}",
                    )
                    for j in range(len(n_splits))
                ]

            def emit_mms(psums, xsubs, ts, kt):
                lhsT = xsubs[kt // ktn_per][
                    :, kt % ktn_per, ts * P : (ts + 1) * P
                ]
                for j, (n0, w) in enumerate(n_splits):
                    nc.tensor.matmul(
                        psums[j][:, :w],
                        lhsT,
                        wt_tiles[kt][:, n0 : n0 + w],
                        start=(kt == 0),
                        stop=(kt == kt_n - 1),
                    )

            def emit_tail(psums, r0):
                osb = out_pool.tile(
                    [P, nc_n], mybir.dt.float16, tag="osb", name=f"osb{r0}"
                )
                for j, (n0, w) in enumerate(n_splits):
                    nc.scalar.copy(osb[:, n0 : n0 + w], psums[j][:, :w])
                nc.sync.dma_start(out[r0 : r0 + P, :], osb[:])

            # ---- dequant prologue: W^T tiles resident in SBUF.
            # The dequant stream is DMA-rate-bound (~2us per kt), so the
            # PE chases it kt-outer with the MMs of the first THREE
            # 128-token tiles: t0/t1 (chunk 0) fully on the main psum
            # gens, t2 (chunk 1) n-splits 0,1 on the 2 spare PSUM banks
            # (its 352-wide split runs as a burst right after).  x sub
            # loads are staggered so early wt DMAs aren't queued behind
            # x bytes that aren't needed until later k-tiles.
            xt_tiles = {0: [None] * ksplit, tok_chunk: [None] * ksplit}
            # sub s of chunk c covers k-tiles 8s..8s+7; its MMs start at
            # kt=8s, so issue its DMA a few k-tiles ahead.
            x_stagger = {
                0: [(0, 0), (tok_chunk, 0)],
                4: [(0, 1)], 5: [(tok_chunk, 1)],
                12: [(0, 2)], 13: [(tok_chunk, 2)],
                20: [(0, 3)], 21: [(tok_chunk, 3)],
            }
            wt_tiles = []
            pro_full = [0, 1]          # t-tiles 0,1 from chunk 0
            pro_psums = [alloc_psums(ts) for ts in pro_full]
            pse = [
                pse_pool.tile(
                    [P, MM_FREE], mybir.dt.float32,
                    tag=f"pse{j}", name=f"pse{j}",
                )
                for j in range(2)
            ]

            def emit_chase(kt):
                for i, ts in enumerate(pro_full):
                    emit_mms(pro_psums[i], xt_tiles[0], ts, kt)
                # partial t2 chase: n-splits 0,1 into the spare banks
                lhsT2 = xt_tiles[tok_chunk][kt // ktn_per][
                    :, kt % ktn_per, 0:P
                ]
                for j in range(2):
                    n0, w = n_splits[j]
                    nc.tensor.matmul(
                        pse[j][:, :w], lhsT2, wt_tiles[kt][:, n0 : n0 + w],
                        start=(kt == 0), stop=(kt == kt_n - 1),
                    )

            for kt in range(kt_n):
                wsb = wsb_pool.tile([P, nc_n], mybir.dt.uint8, tag="wsb")
                nc.sync.dma_start(wsb[:], wpk[kt])
                ssb = ssb_pool.tile([P, nc_n], mybir.dt.float16, tag="ssb")
                nc.sync.dma_start(ssb[:], scl[kt])
                wt = wt_pool.tile([P, nc_n], mybir.dt.float16, tag=f"wt{kt}")
                # lo nibbles in partitions 0-63, hi in 64-127 (host duplicated
                # the bytes into both halves; DVE lanes stay in-partition).
                # Word-wise nibble extraction: process 4 bytes per lane-cycle
                # via a u32 view; the 0x0F0F0F0F mask clears cross-byte bits.
                w32 = wsb[:].bitcast(mybir.dt.uint32)
                nc.vector.tensor_scalar(
                    w32[0:64, :], w32[0:64, :], 0x0F0F0F0F, None,
                    mybir.AluOpType.bitwise_and,
                )
                nc.vector.tensor_scalar(
                    w32[64:P, :], w32[64:P, :], 4, 0x0F0F0F0F,
                    mybir.AluOpType.logical_shift_right,
                    mybir.AluOpType.bitwise_and,
                )
                # (nibble - 8) cast to fp16 on ACT
                nc.scalar.activation(
                    wt[:], wsb[:], mybir.ActivationFunctionType.Identity,
                    bias=-8.0, scale=1.0,
                )
                # x per-(n, group) scale on DVE
                nc.vector.tensor_tensor(
                    wt[:], wt[:], ssb[:], mybir.AluOpType.mult
                )
                wt_tiles.append(wt)
                for c0s, s in x_stagger.get(kt, ()):
                    xt_tiles[c0s][s] = load_x_sub(c0s, s)
                emit_chase(kt)

            for i, ts in enumerate(pro_full):
                emit_tail(pro_psums[i], ts * P)
            # t2's remaining 352-wide split: full-speed burst (wt resident)
            ps2_t2 = psum_pool.tile(
                [P, MM_FREE], mybir.dt.float32, tag="ps2", name="ps2_t2"
            )
            n0_2, w_2 = n_splits[2]
            for kt in range(kt_n):
                nc.tensor.matmul(
                    ps2_t2[:, :w_2],
                    xt_tiles[tok_chunk][kt // ktn_per][:, kt % ktn_per, 0:P],
                    wt_tiles[kt][:, n0_2 : n0_2 + w_2],
                    start=(kt == 0), stop=(kt == kt_n - 1),
                )
            emit_tail([pse[0], pse[1], ps2_t2], tok_chunk)

            # ---- main loop: x^T chunks x W^T -> out rows ----
            for c0 in range(0, t, tok_chunk):
                if c0 in xt_tiles:
                    xsubs = xt_tiles.pop(c0)
                else:
                    xsubs = load_x_chunk(c0)
                for ts in range(tok_chunk // P):
                    if (c0, ts) in ((0, 0), (0, 1), (tok_chunk, 0)):
                        continue  # handled in the prologue
                    psums = alloc_psums(c0 + ts * P)
                    for kt in range(kt_n):
                        emit_mms(psums, xsubs, ts, kt)
                    emit_tail(psums, c0 + ts * P)
    return nc


# ---------------------------------------------------------------------------
def pack_inputs(x, weight_packed, scales, t=T, k=K, nc_n=NC_N, ncores=NCORES):
    """Host-side layout prep (transpose/permute only)."""
    x = np.asarray(x, dtype=np.float16).reshape(t, k)
    wp = np.asarray(weight_packed, dtype=np.uint8)
    sc = np.asarray(scales, dtype=np.float16)
    kt_n = k // P
    ksplit = 4 if kt_n % 4 == 0 else 1
    ktn_per = kt_n // ksplit
    n_chunks = t // TOK_CHUNK

    # xTp[kt*128 + par*64 + j, tt] = x[tt, kt*128 + 2j + par]  (nibble-parity
    # permutation so device-side dequant never needs a transpose), then
    # regroup into per-(chunk, ksplit) sub-tiles that are contiguous per
    # partition: xh[(c*ksplit + s), p, kk*TOK_CHUNK + tt]
    #   = xTp[(s*ktn_per + kk)*128 + p, c*TOK_CHUNK + tt]
    xTp = x.reshape(t, kt_n, 64, 2).transpose(1, 3, 2, 0).reshape(k, t)
    xh = np.ascontiguousarray(
        xTp.reshape(ksplit, ktn_per, P, n_chunks, TOK_CHUNK)
        .transpose(3, 0, 2, 1, 4)
        .reshape(n_chunks * ksplit, P, ktn_per * TOK_CHUNK)
    )

    in_maps = []
    for c in range(ncores):
        n0 = c * nc_n
        wpT = wp[n0 : n0 + nc_n].T  # [k/2, nc_n]
        v = wpT.reshape(kt_n, 64, nc_n)
        wpk = np.empty((kt_n, P, nc_n), dtype=np.uint8)
        wpk[:, 0:64] = v
        wpk[:, 64:P] = v
        sclT = sc[n0 : n0 + nc_n].T  # [kt_n, nc_n]
        scl = np.ascontiguousarray(
            np.broadcast_to(sclT[:, None, :], (kt_n, P, nc_n))
        )
        in_maps.append({"xh": xh, "wpk": wpk, "scl": scl})
    return in_maps


def run(x, weight_packed, scales, trace=False):
    _install_ntff_shim()
    from concourse.bass_utils import run_bass_kernel_spmd

    nc = build_nc()
    _legalize_multi_waits(nc, max_waits=1)
    in_maps = pack_inputs(x, weight_packed, scales)
    # transient NRT device errors (NRT_EXEC_UNIT_UNRECOVERABLE) have been
    # observed to clear on retry; back off briefly between attempts.
    import time as _time

    last_exc = None
    for attempt in range(4):
        try:
            res = run_bass_kernel_spmd(
                nc, in_maps, core_ids=list(range(NCORES)), trace=trace
            )
            break
        except Exception as e:
            last_exc = e
            _time.sleep(15 * (attempt + 1))
    else:
        raise last_exc
    parts = [res.results[c]["out"] for c in range(NCORES)]
    full = np.concatenate(parts, axis=1).reshape(B, S, N)
    return full, res


def kernel(x, weight_packed, scales):
    full, _ = run(x, weight_packed, scales, trace=False)
    return full


if __name__ == "__main__":
    rng = np.random.default_rng(0)
    x = rng.standard_normal((B, S, K)).astype(np.float16)
    wp = rng.integers(0, 256, (N, K // 2)).astype(np.uint8)
    sc = (rng.random((N, K // KT)).astype(np.float16) * np.float16(0.1))
    out = kernel(x, wp, sc)
    print(out.shape, out.dtype)



# revision 21
# speedup vs baseline: 1.0151x; 1.0026x over previous
"""Int4 group-quantized dense MLP matmul on 8 Trainium2 NeuronCores.

Computes out[b,s,n] = sum_k x[b,s,k] * W[n,k] where W is dequantized from
packed int4 (two nibbles per byte, per-128-group fp16 scales).

Strategy (tensor-parallel over N):
  - N=11008 output features sharded 1376 per core across 8 cores.
  - Host does LAYOUT ONLY: transpose x to k-major (with a per-k-tile
    nibble-parity permutation so device-side dequant never needs a
    transpose), transpose/duplicate the packed weight bytes, replicate
    scales. All arithmetic (nibble extract, -8, x scale, matmul) runs on
    device.
  - Device per core: dequantize W^T into SBUF-resident fp16 tiles
    [128 k x 1376 n] (lo nibbles -> partitions 0-63, hi -> 64-127), then
    for each 128-token tile accumulate 32 k-tile matmuls into PSUM fp32
    and write fp16 output rows.
"""

import numpy as np

B, S, K, N = 4, 2048, 4096, 11008
T = B * S                      # 8192 tokens
P = 128                        # partitions
KT = K // P                    # 32 k-tiles (each is one quant group)
NCORES = 8
NC_N = N // NCORES             # 1376 features per core
TOK_CHUNK = 256                # tokens per x DMA chunk
MM_FREE = 512                  # max moving free dim per matmul (PSUM bank)


# ---------------------------------------------------------------------------
# walrus in this container accepts only ONE sync wait per instruction;
# split extras onto same-engine NoOps placed immediately before.
def _legalize_multi_waits(nc, max_waits=1):
    from concourse import mybir

    n_fixed = 0
    for f in nc.m.functions:
        for bb in f.blocks:
            insts = bb.instructions
            i = 0
            while i < len(insts):
                inst = insts[i]
                si = inst.sync_info
                if si is not None and si.on_wait and len(si.on_wait) > max_waits:
                    waits = list(si.on_wait)
                    extra, keep = waits[:-max_waits], waits[-max_waits:]
                    chain = []
                    for j in range(0, len(extra), max_waits):
                        chunk = extra[j : j + max_waits]
                        chain.append(
                            mybir.InstNoOp(
                                name=f"{inst.name}-waitsplit-{j}",
                                engine=inst.engine,
                                bass_nofuse=True,
                                sync_info=mybir.SyncInfo(on_wait=chunk, on_update=[]),
                            )
                        )
                    si.on_wait = keep
                    for k, nop in enumerate(chain):
                        insts.insert(i + k, nop)
                    i += len(chain)
                    n_fixed += 1
                i += 1
    return n_fixed


def _install_ntff_shim():
    """Make trace=True work: register the NTFF profile hook that the agent
    image's antenv lacks, and keep artifacts local."""
    import sys, types

    try:
        import antenv.axon_hooks  # noqa: F401

        return
    except ImportError:
        pass
    try:
        from trn_agent_boot.trn_boot import _ntff_profile_via_ctypes

        hook = _ntff_profile_via_ctypes("/opt/axon/libaxon_pjrt.so")
    except Exception:
        hook = None
    mod = types.ModuleType("antenv.axon_hooks")
    mod.get_axon_ntff_profile_hook = lambda: hook
    mod.set_axon_ntff_profile_hook = lambda h: None
    sys.modules["antenv.axon_hooks"] = mod

    import concourse.bass_utils as bu

    bu.upload_artifacts = lambda tmpdir: "local://" + str(tmpdir)


# ---------------------------------------------------------------------------
def build_nc(t=T, k=K, nc_n=NC_N, tok_chunk=TOK_CHUNK):
    """Build the per-core Bass program (same NEFF on all cores; per-core
    inputs differ). Inputs: xh [t/tok_chunk*ksplit, 128, ktn_per*tok_chunk]
    fp16 (per-partition-contiguous sub-tiles), wpk [kt, 128, nc_n] u8,
    scl [kt, 128, nc_n] fp16. Output: out [t, nc_n] fp16."""
    import concourse.bass as bass
    import concourse.tile as tile
    from concourse import mybir

    kt_n = k // P
    assert t % tok_chunk == 0 and tok_chunk % P == 0
    n_splits = [
        (n0, min(MM_FREE, nc_n - n0)) for n0 in range(0, nc_n, MM_FREE)
    ]

    nc = bass.Bass()
    # const AP for the ACT bias (-8): same pattern as Bass.__init__ consts
    _c = nc.alloc_sbuf_tensor("const-float32-m8", [P, 1], mybir.dt.float32)
    nc.gpsimd.memset(_c.ap(), -8.0)
    nc.const_aps.aps[(mybir.dt.float32, -8.0)] = _c.ap()
    nc.all_engine_barrier()

    ksplit = 4 if kt_n % 4 == 0 else 1
    ktn_per = kt_n // ksplit

    xh = nc.declare_dram_parameter(
        "xh",
        [t // tok_chunk * ksplit, P, ktn_per * tok_chunk],
        mybir.dt.float16,
        isOutput=False,
    )
    wpk = nc.declare_dram_parameter(
        "wpk", [kt_n, P, nc_n], mybir.dt.uint8, isOutput=False
    )
    scl = nc.declare_dram_parameter(
        "scl", [kt_n, P, nc_n], mybir.dt.float16, isOutput=False
    )
    out = nc.declare_dram_parameter("out", [t, nc_n], mybir.dt.float16, isOutput=True)

    with tile.TileContext(nc) as tc:
        with (
            tc.tile_pool(name="wt", bufs=1) as wt_pool,
            tc.tile_pool(name="wsb", bufs=5) as wsb_pool,
            tc.tile_pool(name="ssb", bufs=5) as ssb_pool,
            tc.tile_pool(name="xt", bufs=4) as x_pool,
            tc.tile_pool(name="osb", bufs=3) as out_pool,
            tc.tile_pool(name="psum", bufs=2, space="PSUM") as psum_pool,
            tc.tile_pool(name="pse", bufs=1, space="PSUM") as pse_pool,
        ):
            def load_x_sub(c0, s):
                xs = x_pool.tile(
                    [P, ktn_per, tok_chunk], mybir.dt.float16,
                    tag=f"xt{s}", name=f"xt{s}_{c0}",
                )
                nc.sync.dma_start(
                    xs[:],
                    xh[c0 // tok_chunk * ksplit + s].rearrange(
                        "p (kk tt) -> p kk tt", kk=ktn_per
                    ),
                )
                return xs

            def load_x_chunk(c0):
                """x chunk as `ksplit` sub-tiles (each 4KB-contiguous per
                partition in DRAM) so the first k-tiles land early."""
                return [load_x_sub(c0, s) for s in range(ksplit)]

            def alloc_psums(ts_abs):
                return [
                    psum_pool.tile(
                        [P, MM_FREE], mybir.dt.float32,
                        tag=f"ps{j}", name=f"ps{j}_{ts_abs}",
                    )
                    for j in range(len(n_splits))
                ]

            def emit_mms(psums, xsubs, ts, kt):
                lhsT = xsubs[kt // ktn_per][
                    :, kt % ktn_per, ts * P : (ts + 1) * P
                ]
                for j, (n0, w) in enumerate(n_splits):
                    nc.tensor.matmul(
                        psums[j][:, :w],
                        lhsT,
                        wt_tiles[kt][:, n0 : n0 + w],
                        start=(kt == 0),
                        stop=(kt == kt_n - 1),
                    )

            def emit_tail(psums, r0):
                osb = out_pool.tile(
                    [P, nc_n], mybir.dt.float16, tag="osb", name=f"osb{r0}"
                )
                for j, (n0, w) in enumerate(n_splits):
                    nc.scalar.copy(osb[:, n0 : n0 + w], psums[j][:, :w])
                nc.sync.dma_start(out[r0 : r0 + P, :], osb[:])

            # ---- dequant prologue: W^T tiles resident in SBUF.
            # The dequant stream is DMA-rate-bound (~2us per kt), so the
            # PE chases it kt-outer with the MMs of the first THREE
            # 128-token tiles: t0/t1 (chunk 0) fully on the main psum
            # gens, t2 (chunk 1) n-splits 0,1 on the 2 spare PSUM banks
            # (its 352-wide split runs as a burst right after).  x sub
            # loads are staggered so early wt DMAs aren't queued behind
            # x bytes that aren't needed until later k-tiles.
            xt_tiles = {0: [None] * ksplit, tok_chunk: [None] * ksplit}
            # sub s of chunk c covers k-tiles 8s..8s+7; its MMs start at
            # kt=8s, so issue its DMA a few k-tiles ahead.
            x_stagger = {
                0: [(0, 0)], 1: [(tok_chunk, 0)],
                4: [(0, 1)], 5: [(tok_chunk, 1)],
                12: [(0, 2)], 13: [(tok_chunk, 2)],
                20: [(0, 3)], 21: [(tok_chunk, 3)],
            }
            wt_tiles = []
            pro_full = [0, 1]          # t-tiles 0,1 from chunk 0
            pro_psums = [alloc_psums(ts) for ts in pro_full]
            pse = [
                pse_pool.tile(
                    [P, MM_FREE], mybir.dt.float32,
                    tag=f"pse{j}", name=f"pse{j}",
                )
                for j in range(2)
            ]

            def emit_chase(kt):
                for i, ts in enumerate(pro_full):
                    emit_mms(pro_psums[i], xt_tiles[0], ts, kt)

            def emit_chase2(kt):
                # partial t2 chase: n-splits 0,1 into the spare banks
                # (lags one kt so its x sub DMA doesn't delay early wt tiles)
                lhsT2 = xt_tiles[tok_chunk][kt // ktn_per][
                    :, kt % ktn_per, 0:P
                ]
                for j in range(2):
                    n0, w = n_splits[j]
                    nc.tensor.matmul(
                        pse[j][:, :w], lhsT2, wt_tiles[kt][:, n0 : n0 + w],
                        start=(kt == 0), stop=(kt == kt_n - 1),
                    )

            for kt in range(kt_n):
                wsb = wsb_pool.tile([P, nc_n], mybir.dt.uint8, tag="wsb")
                nc.sync.dma_start(wsb[:], wpk[kt])
                ssb = ssb_pool.tile([P, nc_n], mybir.dt.float16, tag="ssb")
                nc.sync.dma_start(ssb[:], scl[kt])
                wt = wt_pool.tile([P, nc_n], mybir.dt.float16, tag=f"wt{kt}")
                # lo nibbles in partitions 0-63, hi in 64-127 (host duplicated
                # the bytes into both halves; DVE lanes stay in-partition).
                # Word-wise nibble extraction: process 4 bytes per lane-cycle
                # via a u32 view; the 0x0F0F0F0F mask clears cross-byte bits.
                w32 = wsb[:].bitcast(mybir.dt.uint32)
                nc.vector.tensor_scalar(
                    w32[0:64, :], w32[0:64, :], 0x0F0F0F0F, None,
                    mybir.AluOpType.bitwise_and,
                )
                nc.vector.tensor_scalar(
                    w32[64:P, :], w32[64:P, :], 4, 0x0F0F0F0F,
                    mybir.AluOpType.logical_shift_right,
                    mybir.AluOpType.bitwise_and,
                )
                # (nibble - 8) cast to fp16 on ACT
                nc.scalar.activation(
                    wt[:], wsb[:], mybir.ActivationFunctionType.Identity,
                    bias=-8.0, scale=1.0,
                )
                # x per-(n, group) scale on DVE
                nc.vector.tensor_tensor(
                    wt[:], wt[:], ssb[:], mybir.AluOpType.mult
                )
                wt_tiles.append(wt)
                for c0s, s in x_stagger.get(kt, ()):
                    xt_tiles[c0s][s] = load_x_sub(c0s, s)
                emit_chase(kt)
                if kt >= 1:
                    emit_chase2(kt - 1)

            emit_chase2(kt_n - 1)
            for i, ts in enumerate(pro_full):
                emit_tail(pro_psums[i], ts * P)
            # t2's remaining 352-wide split: full-speed burst (wt resident)
            ps2_t2 = psum_pool.tile(
                [P, MM_FREE], mybir.dt.float32, tag="ps2", name="ps2_t2"
            )
            n0_2, w_2 = n_splits[2]
            for kt in range(kt_n):
                nc.tensor.matmul(
                    ps2_t2[:, :w_2],
                    xt_tiles[tok_chunk][kt // ktn_per][:, kt % ktn_per, 0:P],
                    wt_tiles[kt][:, n0_2 : n0_2 + w_2],
                    start=(kt == 0), stop=(kt == kt_n - 1),
                )
            emit_tail([pse[0], pse[1], ps2_t2], tok_chunk)

            # ---- main loop: x^T chunks x W^T -> out rows ----
            for c0 in range(0, t, tok_chunk):
                if c0 in xt_tiles:
                    xsubs = xt_tiles.pop(c0)
                else:
                    xsubs = load_x_chunk(c0)
                for ts in range(tok_chunk // P):
                    if (c0, ts) in ((0, 0), (0, 1), (tok_chunk, 0)):
                        continue  # handled in the prologue
                    psums = alloc_psums(c0 + ts * P)
                    for kt in range(kt_n):
                        emit_mms(psums, xsubs, ts, kt)
                    emit_tail(psums, c0 + ts * P)
    return nc


# ---------------------------------------------------------------------------
def pack_inputs(x, weight_packed, scales, t=T, k=K, nc_n=NC_N, ncores=NCORES):
    """Host-side layout prep (transpose/permute only)."""
    x = np.asarray(x, dtype=np.float16).reshape(t, k)
    wp = np.asarray(weight_packed, dtype=np.uint8)
    sc = np.asarray(scales, dtype=np.float16)
    kt_n = k // P
    ksplit = 4 if kt_n % 4 == 0 else 1
    ktn_per = kt_n // ksplit
    n_chunks = t // TOK_CHUNK

    # xTp[kt*128 + par*64 + j, tt] = x[tt, kt*128 + 2j + par]  (nibble-parity
    # permutation so device-side dequant never needs a transpose), then
    # regroup into per-(chunk, ksplit) sub-tiles that are contiguous per
    # partition: xh[(c*ksplit + s), p, kk*TOK_CHUNK + tt]
    #   = xTp[(s*ktn_per + kk)*128 + p, c*TOK_CHUNK + tt]
    xTp = x.reshape(t, kt_n, 64, 2).transpose(1, 3, 2, 0).reshape(k, t)
    xh = np.ascontiguousarray(
        xTp.reshape(ksplit, ktn_per, P, n_chunks, TOK_CHUNK)
        .transpose(3, 0, 2, 1, 4)
        .reshape(n_chunks * ksplit, P, ktn_per * TOK_CHUNK)
    )

    in_maps = []
    for c in range(ncores):
        n0 = c * nc_n
        wpT = wp[n0 : n0 + nc_n].T  # [k/2, nc_n]
        v = wpT.reshape(kt_n, 64, nc_n)
        wpk = np.empty((kt_n, P, nc_n), dtype=np.uint8)
        wpk[:, 0:64] = v
        wpk[:, 64:P] = v
        sclT = sc[n0 : n0 + nc_n].T  # [kt_n, nc_n]
        scl = np.ascontiguousarray(
            np.broadcast_to(sclT[:, None, :], (kt_n, P, nc_n))
        )
        in_maps.append({"xh": xh, "wpk": wpk, "scl": scl})
    return in_maps


def run(x, weight_packed, scales, trace=False):
    _install_ntff_shim()
    from concourse.bass_utils import run_bass_kernel_spmd

    nc = build_nc()
    _legalize_multi_waits(nc, max_waits=1)
    in_maps = pack_inputs(x, weight_packed, scales)
    # transient NRT device errors (NRT_EXEC_UNIT_UNRECOVERABLE) have been
    # observed to clear on retry; back off briefly between attempts.
    import time as _time

    last_exc = None
    for attempt in range(4):
        try:
            res = run_bass_kernel_spmd(
                nc, in_maps, core_ids=list(range(NCORES)), trace=trace
            )
            break
        except Exception as e:
            last_exc = e
            _time.sleep(15 * (attempt + 1))
    else:
        raise last_exc
    parts = [res.results[c]["out"] for c in range(NCORES)]
    full = np.concatenate(parts, axis=1).reshape(B, S, N)
    return full, res


def kernel(x, weight_packed, scales):
    full, _ = run(x, weight_packed, scales, trace=False)
    return full


if __name__ == "__main__":
    rng = np.random.default_rng(0)
    x = rng.standard_normal((B, S, K)).astype(np.float16)
    wp = rng.integers(0, 256, (N, K // 2)).astype(np.uint8)
    sc = (rng.random((N, K // KT)).astype(np.float16) * np.float16(0.1))
    out = kernel(x, wp, sc)
    print(out.shape, out.dtype)

